# revision 3
# baseline (speedup 1.0000x reference)
"""ComplexLSTM Trainium2 kernel.

Problem: B=32, I=128, H=256, T=2048. Four independent LSTM scans
(real/imag weights x real/imag inputs) combined into a complex output
(B, H, T) complex64.

Sharding: data-parallel over batch across 8 cores (4 rows each); each
core runs all four scans for its batch slice, organized as two "chains"
that share a recurrent weight matrix (Whh_r / Whh_i), so each chain's
per-step recurrent matmul streams its weight once for both scans.

Per-step math (PyTorch gate order i, f, g, o; g rows pre-scaled by 2 on
host so a single sigmoid covers all gates: tanh(x) = 2*sigmoid(2x)-1):
  gates = gx_t + h @ Whh.T        (PE; fp32r; gx & bias folded in)
  S     = sigmoid(gates)          (ACT, one instruction)
  c'    = S_f*c + 2*S_i*S_g - S_i (DVE)
  h     = 2*S_o*sigmoid(2c') - S_o (ACT + DVE)
h is transposed each step on the PE (identity-matmul transpose) to feed
the next step's stationary operand.
"""

import numpy as np
from contextlib import ExitStack

import concourse.bass as bass
import concourse.tile as tile
import concourse.mybir as mybir
from concourse import bacc
from concourse.bass import ds
from concourse.bass_utils import run_bass_kernel_spmd
from concourse.masks import make_identity

B, I, H = 32, 128, 256
G = 4 * H            # 1024
NCORES = 8
BL = B // NCORES     # 4 batch rows per core
ROWS = 2 * BL        # 8 rows per chain (2 scans x 4 batch)
U = 8                # steps per For_i iteration

f32 = mybir.dt.float32
f32r = mybir.dt.float32r
SIG = mybir.ActivationFunctionType.Sigmoid
MULT = mybir.AluOpType.mult
SUB = mybir.AluOpType.subtract
TANH = mybir.ActivationFunctionType.Tanh


def r(ap):
    return ap.bitcast(f32r)


def build_program(T):
    import os
    skip1 = os.environ.get("K_SKIP1") == "1"
    skip2 = os.environ.get("K_SKIP2") == "1"
    skip3 = os.environ.get("K_SKIP3") == "1"
    TC = T // 128      # phase-1 tiles per (scan, b)
    nc = bacc.Bacc("TRN2", target_bir_lowering=False, debug=False,
                   num_devices=NCORES)

    xr = nc.declare_dram_parameter("xr", [BL, I, T], f32r, isOutput=False)
    xi = nc.declare_dram_parameter("xi", [BL, I, T], f32r, isOutput=False)
    wih = nc.declare_dram_parameter("wih", [2, I, G], f32r, isOutput=False)
    whh = nc.declare_dram_parameter("whh", [2, H, G], f32r, isOutput=False)
    bia = nc.declare_dram_parameter("bias", [2, G], f32, isOutput=False)
    sgn = nc.declare_dram_parameter("sgn", [ROWS, 1], f32, isOutput=False)
    id8 = nc.declare_dram_parameter("id8", [ROWS, ROWS], f32r, isOutput=False)
    out = nc.declare_dram_parameter("out", [BL, H, 2 * T], f32, isOutput=True)

    # staging: gx per chain [T, ROWS, G]; h combine output [T, ROWS, H]
    gxst = [nc.dram_tensor(f"gx_stage{c}", [T, ROWS, G], f32r) for c in range(2)]
    hst = nc.dram_tensor("h_stage", [T, ROWS, H], f32)

    # chain c, slot s -> input tensor (chain0=Wr: xr,xi ; chain1=Wi: xi,xr)
    def xsrc(c, s):
        return (xr if s == 0 else xi) if c == 0 else (xi if s == 0 else xr)

    with tile.TileContext(nc) as tc, ExitStack() as top:
        consts = top.enter_context(tc.tile_pool(name="consts", bufs=1))

        whh_sb = [[consts.tile([128, G], f32r, name=f"whh{c}{k}", tag=f"whh{c}{k}") for k in range(2)]
                  for c in range(2)]
        for c in range(2):
            for k in range(2):
                nc.sync.dma_start(out=whh_sb[c][k],
                                  in_=whh[c, k * 128:(k + 1) * 128, :])

        ident8 = consts.tile([ROWS, ROWS], f32r, tag="ident8")
        nc.sync.dma_start(out=ident8, in_=id8[:, :])
        ident8f = consts.tile([ROWS, ROWS], f32, tag="ident8f")
        make_identity(nc, ident8f)
        sgn_sb = consts.tile([ROWS, 1], f32, tag="sgn_sb")
        nc.sync.dma_start(out=sgn_sb, in_=sgn[:, :])

        # ---------------- phase 1: gx = x @ WihT (+bias) ----------------
        with ExitStack() as p1:
          if not skip1:
            p1c = p1.enter_context(tc.tile_pool(name="p1c", bufs=1))
            xp = p1.enter_context(tc.tile_pool(name="xp", bufs=4))
            gp = p1.enter_context(tc.tile_pool(name="gp", bufs=2, space="PSUM"))
            gs = p1.enter_context(tc.tile_pool(name="gs", bufs=4))

            wih_sb = [p1c.tile([I, G], f32r, name=f"wih{c}", tag=f"wih{c}") for c in range(2)]
            bia_sb = [p1c.tile([128, G], f32, name=f"bia{c}", tag=f"bia{c}") for c in range(2)]
            for c in range(2):
                nc.sync.dma_start(out=wih_sb[c], in_=wih[c])
                bsrc = bia[c:c + 1, :]
                nc.sync.dma_start(
                    out=bia_sb[c],
                    in_=bass.AP(tensor=bsrc.tensor, offset=bsrc.offset,
                                ap=[[0, 128]] + list(bsrc.ap[-1:])))

            for c in range(2):
                for s in range(2):
                    src = xsrc(c, s)
                    for b in range(BL):
                        for t in range(TC):
                            xt = xp.tile([I, 128], f32r, tag="xt")
                            nc.sync.dma_start(
                                out=xt, in_=src[b, :, t * 128:(t + 1) * 128])
                            ps = gp.tile([128, G], f32, tag="ps")
                            for n in range(2):
                                sl = ds(n * 512, 512)
                                nc.tensor.matmul(ps[:, sl], xt,
                                                 wih_sb[c][:, sl],
                                                 start=True, stop=True)
                            gt = gs.tile([128, G], f32r, tag="gt")
                            nc.vector.tensor_add(
                                gt, ps, bia_sb[c])
                            nc.sync.dma_start(
                                out=gxst[c][t * 128:(t + 1) * 128,
                                            s * BL + b, :],
                                in_=gt.rearrange("p (o g) -> p o g", o=1))

        # ---------------- phase 2: the recurrence ----------------
        with ExitStack() as p2:
          if not skip2:
            st8 = p2.enter_context(tc.tile_pool(name="st8", bufs=1))
            gxp = p2.enter_context(tc.tile_pool(name="gxp", bufs=2))
            spl = p2.enter_context(tc.tile_pool(name="spl", bufs=2))
            tmp = p2.enter_context(tc.tile_pool(name="tmp", bufs=2))
            hpl = p2.enter_context(tc.tile_pool(name="hpl", bufs=3))
            stg = p2.enter_context(tc.tile_pool(name="stg", bufs=2))
            psA = p2.enter_context(tc.tile_pool(name="psA", bufs=2, space="PSUM"))
            psB = p2.enter_context(tc.tile_pool(name="psB", bufs=1, space="PSUM"))
            psT = p2.enter_context(tc.tile_pool(name="psT", bufs=2, space="PSUM"))

            # persistent state (ping-pong on step parity)
            hT = [[[st8.tile([128, ROWS], f32r, name=f"hT{c}{p}{k}", tag=f"hT{c}{p}{k}")
                    for k in range(2)] for p in range(2)] for c in range(2)]
            cst = [[st8.tile([ROWS, H], f32, name=f"c{c}{p}", tag=f"c{c}{p}")
                    for p in range(2)] for c in range(2)]
            for c in range(2):
                for p in range(2):
                    nc.vector.memset(cst[c][p], 0.0)
                    for k in range(2):
                        nc.vector.memset(hT[c][p][k].bitcast(f32), 0.0)

            stt = nc.vector.scalar_tensor_tensor
            ADD = mybir.AluOpType.add

            with tc.For_i(0, T, U, staggered_reset=True) as iv:
                gxch = [gxp.tile([ROWS, U, G], f32r, name=f"gx{c}", tag=f"gx{c}")
                        for c in range(2)]
                for c in range(2):
                    nc.sync.dma_start(
                        out=gxch[c],
                        in_=gxst[c][ds(iv, U), :, :].rearrange("u p g -> p u g"))
                st = stg.tile([ROWS, U, H], f32, tag="st")
                for k in range(U):
                    pp = k % 2
                    # wave 0: all matmuls (both chains)
                    gates, S, slc = [], [], {}
                    for c in range(2):
                        gpool = psA if c == 0 else psB
                        g_ = gpool.tile([ROWS, G], f32, name=f"gates{c}",
                                        tag=f"gates{c}")
                        gates.append(g_)
                        for n in range(2):
                            sl = ds(n * 512, 512)
                            nc.tensor.matmul(g_[:, sl], hT[c][pp][0],
                                             whh_sb[c][0][:, sl],
                                             start=True, stop=False)
                            nc.tensor.matmul(g_[:, sl], hT[c][pp][1],
                                             whh_sb[c][1][:, sl],
                                             start=False, stop=False)
                            nc.tensor.matmul(g_[:, sl], ident8,
                                             gxch[c][:, k, sl],
                                             start=False, stop=True)
                    # wave 1: sigmoids
                    for c in range(2):
                        S_ = spl.tile([ROWS, G], f32, name=f"S{c}", tag=f"S{c}")
                        nc.scalar.activation(S_, gates[c], SIG)
                        S.append(S_)
                        slc[c] = (S_[:, 0:H], S_[:, H:2 * H],
                                  S_[:, 2 * H:3 * H], S_[:, 3 * H:4 * H])
                    # wave 2: c update. GPSIMD (idle otherwise) takes the
                    # two independent products; DVE takes the dependent ops.
                    pt_, ut_, vt_ = {}, {}, {}
                    for c in range(2):
                        Si, Sf, Sg, So = slc[c]
                        p_ = tmp.tile([ROWS, H], f32, name=f"p{c}", tag=f"p{c}")
                        nc.gpsimd.tensor_mul(p_, Si, Sg)
                        pt_[c] = p_
                        v = tmp.tile([ROWS, H], f32, name=f"v{c}", tag=f"v{c}")
                        nc.gpsimd.tensor_mul(v, Sf, cst[c][pp])
                        vt_[c] = v
                    for c in range(2):
                        Si, Sf, Sg, So = slc[c]
                        u = tmp.tile([ROWS, H], f32, name=f"u{c}", tag=f"u{c}")
                        stt(out=u, in0=pt_[c], scalar=2.0, in1=Si,
                            op0=MULT, op1=SUB)
                        ut_[c] = u
                    sct = {}
                    for c in range(2):
                        cn = cst[c][1 - pp]
                        nc.vector.tensor_add(cn, ut_[c], vt_[c])
                        # wave 3: tanh(c) right behind the add, per chain
                        tc_ = tmp.tile([ROWS, H], f32, name=f"tc{c}", tag=f"tc{c}")
                        nc.scalar.activation(tc_, cn, TANH)
                        sct[c] = tc_
                    # wave 4: h = sig(o) * tanh(c)
                    hcur = []
                    for c in range(2):
                        Si, Sf, Sg, So = slc[c]
                        h = hpl.tile([ROWS, H], f32, name=f"h{c}", tag=f"h{c}")
                        nc.vector.tensor_mul(h, So, sct[c])
                        hcur.append(h)
                    # wave 5: transposes + state copies
                    for c in range(2):
                        for kh in range(2):
                            pt = psT.tile([128, ROWS], f32, tag="pt")
                            nc.tensor.transpose(
                                pt, hcur[c][:, kh * 128:(kh + 1) * 128], ident8f)
                            nc.vector.tensor_copy(hT[c][1 - pp][kh], pt)
                    # combine: L_r = scan0 - scan1 (rows 0:4),
                    #          L_i = scan2 + scan3 (rows 4:8):
                    # st = hcur[1] * sgn + hcur[0], sgn = [-1]*4 + [1]*4
                    stt(out=st[:, k, :], in0=hcur[1], scalar=sgn_sb,
                        in1=hcur[0], op0=MULT, op1=ADD)
                nc.sync.dma_start(
                    out=hst[ds(iv, U), :, :].rearrange("u p h -> p u h"),
                    in_=st)

        # ---------------- phase 3: transpose to (b, h, t), interleave ----
        with ExitStack() as p3:
          if not skip3:
            p3c = p3.enter_context(tc.tile_pool(name="p3c", bufs=1))
            lp = p3.enter_context(tc.tile_pool(name="lp", bufs=4))
            tp = p3.enter_context(tc.tile_pool(name="tp", bufs=4, space="PSUM"))
            op = p3.enter_context(tc.tile_pool(name="op", bufs=4))

            id128 = p3c.tile([128, 128], f32, tag="id128")
            make_identity(nc, id128)

            for b in range(BL):
                for t in range(TC):
                    lr = lp.tile([128, H], f32, tag="lr")
                    li = lp.tile([128, H], f32, tag="li")
                    nc.sync.dma_start(
                        out=lr, in_=hst[t * 128:(t + 1) * 128, b, :])
                    nc.sync.dma_start(
                        out=li, in_=hst[t * 128:(t + 1) * 128, BL + b, :])
                    for hb in range(2):
                        hsl = ds(hb * 128, 128)
                        ptr = tp.tile([128, 128], f32, tag="ptr")
                        nc.tensor.transpose(ptr, lr[:, hsl], id128)
                        pti = tp.tile([128, 128], f32, tag="pti")
                        nc.tensor.transpose(pti, li[:, hsl], id128)
                        ot = op.tile([128, 256], f32, tag="ot")
                        otv = ot.rearrange("p (t two) -> p t two", two=2)
                        nc.vector.tensor_copy(otv[:, :, 0], ptr)
                        nc.vector.tensor_copy(otv[:, :, 1], pti)
                        nc.sync.dma_start(
                            out=out[b, hsl, ds(2 * t * 128, 256)], in_=ot)

    nc.compile()
    return nc


_CACHE = {}
LAST_RES = None


def get_program(T):
    if T not in _CACHE:
        _CACHE[T] = build_program(T)
    return _CACHE[T]


def _pack_weights(Wih, Whh, bih, bhh):
    Wih = np.array(Wih, dtype=np.float32, copy=True)
    Whh = np.array(Whh, dtype=np.float32, copy=True)
    b = (np.asarray(bih) + np.asarray(bhh)).astype(np.float32)
    # pre-scale g gate (rows 2H:3H) by 2 so sigmoid(2g) gives tanh via 2s-1
    Wih[2 * H:3 * H] *= 2.0
    Whh[2 * H:3 * H] *= 2.0
    b[2 * H:3 * H] *= 2.0
    return np.ascontiguousarray(Wih.T), np.ascontiguousarray(Whh.T), b


def kernel(x_real, x_imag, Wih_r, Whh_r, bih_r, bhh_r,
           Wih_i, Whh_i, bih_i, bhh_i):
    x_real = np.asarray(x_real, dtype=np.float32)
    x_imag = np.asarray(x_imag, dtype=np.float32)
    T = x_real.shape[2]
    nc = get_program(T)

    wihT_r, whhT_r, b_r = _pack_weights(Wih_r, Whh_r, bih_r, bhh_r)
    wihT_i, whhT_i, b_i = _pack_weights(Wih_i, Whh_i, bih_i, bhh_i)
    wih_p = np.ascontiguousarray(np.stack([wihT_r, wihT_i]))
    whh_p = np.ascontiguousarray(np.stack([whhT_r, whhT_i]))
    bia_p = np.ascontiguousarray(np.stack([b_r, b_i]))
    sgn_p = np.array([[-1.0]] * BL + [[1.0]] * BL, dtype=np.float32)
    id8_p = np.eye(ROWS, dtype=np.float32)

    in_maps = []
    for c in range(NCORES):
        sl = slice(c * BL, (c + 1) * BL)
        in_maps.append({
            "xr": np.ascontiguousarray(x_real[sl]),
            "xi": np.ascontiguousarray(x_imag[sl]),
            "wih": wih_p, "whh": whh_p, "bias": bia_p,
            "sgn": sgn_p, "id8": id8_p,
        })
    import os
    trace = os.environ.get("K_TRACE") == "1"
    res = run_bass_kernel_spmd(nc, in_maps, list(range(NCORES)), trace=trace)
    global LAST_RES
    LAST_RES = res
    parts = []
    for c in range(NCORES):
        o = np.ascontiguousarray(res.results[c]["out"])  # [BL, H, 2T] f32
        parts.append(o.view(np.complex64))               # [BL, H, T]
    return np.concatenate(parts, axis=0)



# revision 6
# speedup vs baseline: 1.3100x; 1.3100x over previous
"""ComplexLSTM Trainium2 kernel.

Problem: B=32, I=128, H=256, T=2048. Four independent LSTM scans
(real/imag weights x real/imag inputs) combined into a complex output
(B, H, T) complex64.

Sharding: data-parallel over batch across 8 cores (4 rows each); each
core runs all four scans for its batch slice, organized as two "chains"
that share a recurrent weight matrix (Whh_r / Whh_i).

Phase-2 step layout (per chain, rows R=8 = 2 slots x 4 batch):
  gates PSUM tile [40, 512], one bank: rows 0:8 = gates [i|f],
  rows 32:40 = gates [g|o] (matmul col-tile bases must be 0/32/64).
  gx(+bias) is preloaded into the bank by a scatter-identity matmul
  (K=16 -> M=40), then 4 accumulating matmuls (2 h-chunks x 2 slices)
  add h @ Whh.T. One sigmoid covers all gates (g rows pre-scaled by 2
  on host: tanh(x) = 2*sigmoid(2x)-1).
  c' = Sf*c + 2*Si*Sg - Si (GPSIMD products, DVE combine)
  h  = So * tanh(c')        (ACT + DVE)
  h is transposed on the PE (4 small transposes -> one [128,32] PSUM
  tile) and copied to SBUF with a single ACT copy as next step's
  stationary. The scatter for step k+1 is issued before step k's
  transposes so the PE has work while the elementwise tail drains.
  The real/imag combine is deferred to phase 3.
"""

import numpy as np
from contextlib import ExitStack

import concourse.bass as bass
import concourse.tile as tile
import concourse.mybir as mybir
from concourse import bacc
from concourse.bass import ds
from concourse.bass_utils import run_bass_kernel_spmd
from concourse.masks import make_identity

B, I, H = 32, 128, 256
G = 4 * H            # 1024
NCORES = 8
BL = B // NCORES     # 4 batch rows per core
R = 2 * BL           # 8 rows per chain (2 slots x 4 batch)
U = 8                # steps per For_i iteration

f32 = mybir.dt.float32
f32r = mybir.dt.float32r
bf16 = mybir.dt.bfloat16
SIG = mybir.ActivationFunctionType.Sigmoid
TANH = mybir.ActivationFunctionType.Tanh
MULT = mybir.AluOpType.mult
SUB = mybir.AluOpType.subtract


def build_program(T):
    import os
    skip1 = os.environ.get("K_SKIP1") == "1"
    skip2 = os.environ.get("K_SKIP2") == "1"
    skip3 = os.environ.get("K_SKIP3") == "1"
    TC = T // 128      # phase-1/3 tiles per (scan, b)
    nc = bacc.Bacc("TRN2", target_bir_lowering=False, debug=False,
                   num_devices=NCORES)

    xr = nc.declare_dram_parameter("xr", [BL, I, T], f32r, isOutput=False)
    xi = nc.declare_dram_parameter("xi", [BL, I, T], f32r, isOutput=False)
    wih = nc.declare_dram_parameter("wih", [2, I, G], f32r, isOutput=False)
    whh = nc.declare_dram_parameter("whh", [2, H, G], bf16, isOutput=False)
    bia = nc.declare_dram_parameter("bias", [2, G], f32, isOutput=False)
    scat = nc.declare_dram_parameter("scat", [2 * R, 40], f32r, isOutput=False)
    out = nc.declare_dram_parameter("out", [BL, H, 2 * T], f32, isOutput=True)

    # staging: gx per chain [T, 16, 512] (rows 0:8 = [i|f], 8:16 = [g|o]);
    # raw h per chain [T, R, H]
    gxst = [nc.dram_tensor(f"gx_stage{c}", [T, 2 * R, 512], f32r)
            for c in range(2)]
    hst = [nc.dram_tensor(f"h_stage{c}", [T, R, H], f32) for c in range(2)]

    # chain c, slot s -> input tensor (chain0=Wr: xr,xi ; chain1=Wi: xi,xr)
    def xsrc(c, s):
        return (xr if s == 0 else xi) if c == 0 else (xi if s == 0 else xr)

    with tile.TileContext(nc) as tc, ExitStack() as top:
        consts = top.enter_context(tc.tile_pool(name="consts", bufs=1))

        whh_sb = [[consts.tile([128, G], bf16, name=f"whh{c}{k}",
                               tag=f"whh{c}{k}") for k in range(2)]
                  for c in range(2)]
        for c in range(2):
            for k in range(2):
                nc.sync.dma_start(out=whh_sb[c][k],
                                  in_=whh[c, k * 128:(k + 1) * 128, :])

        scat_sb = consts.tile([2 * R, 40], f32r, tag="scat_sb")
        nc.sync.dma_start(out=scat_sb, in_=scat[:, :])
        id8f = consts.tile([R, R], f32, tag="id8f")
        make_identity(nc, id8f)

        # ---------------- phase 1: gx = x @ WihT + bias ----------------
        with ExitStack() as p1:
          if not skip1:
            p1c = p1.enter_context(tc.tile_pool(name="p1c", bufs=1))
            xp = p1.enter_context(tc.tile_pool(name="xp", bufs=4))
            gp = p1.enter_context(tc.tile_pool(name="gp", bufs=2, space="PSUM"))
            gs = p1.enter_context(tc.tile_pool(name="gs", bufs=4))

            wih_sb = [p1c.tile([I, G], f32r, name=f"wih{c}", tag=f"wih{c}")
                      for c in range(2)]
            bia_sb = [p1c.tile([128, G], f32, name=f"bia{c}", tag=f"bia{c}")
                      for c in range(2)]
            for c in range(2):
                nc.sync.dma_start(out=wih_sb[c], in_=wih[c])
                bsrc = bia[c:c + 1, :]
                nc.sync.dma_start(
                    out=bia_sb[c],
                    in_=bass.AP(tensor=bsrc.tensor, offset=bsrc.offset,
                                ap=[[0, 128]] + list(bsrc.ap[-1:])))

            for c in range(2):
                for s in range(2):
                    src = xsrc(c, s)
                    for b in range(BL):
                        row = s * BL + b
                        for t in range(TC):
                            xt = xp.tile([I, 128], f32r, tag="xt")
                            nc.sync.dma_start(
                                out=xt, in_=src[b, :, t * 128:(t + 1) * 128])
                            ps = gp.tile([128, G], f32, tag="ps")
                            for n in range(2):
                                sl = ds(n * 512, 512)
                                nc.tensor.matmul(ps[:, sl], xt,
                                                 wih_sb[c][:, sl],
                                                 start=True, stop=True)
                            gt = gs.tile([128, G], f32r, tag="gt")
                            nc.vector.tensor_add(gt, ps, bia_sb[c])
                            tsl = ds(t * 128, 128)
                            nc.sync.dma_start(
                                out=gxst[c][tsl, row, :], in_=gt[:, 0:512])
                            nc.sync.dma_start(
                                out=gxst[c][tsl, R + row, :],
                                in_=gt[:, 512:1024])

        # ---------------- phase 2: the recurrence ----------------
        with ExitStack() as p2:
          if not skip2:
            st8 = p2.enter_context(tc.tile_pool(name="st8", bufs=1))
            gxp = p2.enter_context(tc.tile_pool(name="gxp", bufs=2))
            spl = p2.enter_context(tc.tile_pool(name="spl", bufs=2))
            tmp = p2.enter_context(tc.tile_pool(name="tmp", bufs=2))
            stg = p2.enter_context(tc.tile_pool(name="stg", bufs=2))
            psA = p2.enter_context(tc.tile_pool(name="psA", bufs=2, space="PSUM"))
            psB = p2.enter_context(tc.tile_pool(name="psB", bufs=2, space="PSUM"))
            psT = p2.enter_context(tc.tile_pool(name="psT", bufs=2, space="PSUM"))

            # persistent state (ping-pong on step parity)
            # hTn[p]: [128, 32] cols (c*2+hb)*8 .. +8 = h[c]^T chunk hb
            # c state lives at base partition 32 (rows 32:40) to match the
            # [f|o] gate zone
            hTn = [st8.tile([128, 4 * R], bf16, name=f"hTn{p}", tag=f"hTn{p}")
                   for p in range(2)]
            cst = [[st8.tile([40, H], f32, name=f"c{c}{p}", tag=f"c{c}{p}")
                    for p in range(2)] for c in range(2)]
            for p in range(2):
                nc.vector.memset(hTn[p], 0.0)
                for c in range(2):
                    nc.vector.memset(cst[c][p], 0.0)

            stt = nc.vector.scalar_tensor_tensor

            def hT_sl(p, c, hb):
                j = (c * 2 + hb) * R
                return hTn[p][:, j:j + R]

            with tc.For_i(0, T, U, staggered_reset=True) as iv:
                gxch = [gxp.tile([2 * R, U, 512], f32r, name=f"gx{c}",
                                 tag=f"gx{c}") for c in range(2)]
                for c in range(2):
                    nc.sync.dma_start(
                        out=gxch[c],
                        in_=gxst[c][ds(iv, U), :, :].rearrange(
                            "u p g -> p u g"))
                st = [stg.tile([R, U, H], f32, name=f"st{c}", tag=f"st{c}")
                      for c in range(2)]

                def scatter(c, kk):
                    pool = psA if c == 0 else psB
                    g_ = pool.tile([40, 512], f32, name=f"G{c}", tag=f"G{c}")
                    nc.tensor.matmul(g_, scat_sb, gxch[c][:, kk, :],
                                     start=True, stop=True)
                    return g_

                Gcur = None
                for k in range(U):
                    pp = k % 2
                    if k == 0:
                        Gcur = [scatter(c, 0) for c in range(2)]
                    # gate matmuls accumulate onto gx
                    for c in range(2):
                        for s in range(2):
                            for kc in range(2):
                                nc.tensor.matmul(
                                    Gcur[c][s * 32:s * 32 + R, :],
                                    hT_sl(pp, c, kc),
                                    whh_sb[c][kc][:, s * 512:(s + 1) * 512],
                                    start=False, stop=True,
                                    skip_group_check=True)
                    # prefetch next step's gx into fresh psum banks
                    Gnext = ([scatter(c, k + 1) for c in range(2)]
                             if k + 1 < U else None)
                    # sigmoids (gate cols permuted on host to [i g | f o]:
                    # zone0 rows 0:8 = i|g, zone32 rows 32:40 = f|o)
                    S, slc = [], {}
                    for c in range(2):
                        S_ = spl.tile([40, 512], f32, name=f"S{c}",
                                      tag=f"S{c}")
                        nc.scalar.activation(S_, Gcur[c], SIG)
                        S.append(S_)
                        slc[c] = (S_[0:R, 0:256], S_[32:32 + R, 0:256],
                                  S_[0:R, 256:512], S_[32:32 + R, 256:512])
                    # c update: GPSIMD takes the two independent products
                    # (each zone-homogeneous), DVE the dependent ops
                    pt_, vt_ = {}, {}
                    for c in range(2):
                        Si, Sf, Sg, So = slc[c]
                        p_ = tmp.tile([R, H], f32, name=f"p{c}", tag=f"p{c}")
                        nc.gpsimd.tensor_mul(p_, Si, Sg)
                        pt_[c] = p_
                        v_ = tmp.tile([R, H], f32, name=f"v{c}", tag=f"v{c}")
                        nc.gpsimd.tensor_mul(v_, Sf, cst[c][pp][32:40, :])
                        vt_[c] = v_
                    ut_ = {}
                    for c in range(2):
                        Si = slc[c][0]
                        u_ = tmp.tile([R, H], f32, name=f"u{c}", tag=f"u{c}")
                        stt(out=u_, in0=pt_[c], scalar=2.0, in1=Si,
                            op0=MULT, op1=SUB)
                        ut_[c] = u_
                    sct = {}
                    for c in range(2):
                        cn = cst[c][1 - pp][32:40, :]
                        nc.vector.tensor_add(cn, ut_[c], vt_[c])
                        tc_ = tmp.tile([40, H], f32, name=f"tc{c}",
                                       tag=f"tc{c}")
                        nc.scalar.activation(tc_[32:40, :], cn, TANH)
                        sct[c] = tc_[32:40, :]
                    # h = sig(o) * tanh(c), written straight into the
                    # store buffer
                    for c in range(2):
                        So = slc[c][3]
                        nc.vector.tensor_mul(st[c][:, k, :], So, sct[c])
                    # transposes into one PSUM tile + single copy to SBUF
                    ptt = psT.tile([128, 4 * R], f32, tag="ptt")
                    for c in range(2):
                        for hb in range(2):
                            j = (c * 2 + hb) * R
                            nc.tensor.transpose(
                                ptt[:, j:j + R],
                                st[c][:, k, hb * 128:(hb + 1) * 128], id8f)
                    nc.scalar.copy(hTn[1 - pp], ptt)
                    Gcur = Gnext
                for c in range(2):
                    nc.sync.dma_start(
                        out=hst[c][ds(iv, U), :, :].rearrange("u p h -> p u h"),
                        in_=st[c])

        # ------- phase 3: combine, transpose to (b, h, t), interleave -----
        with ExitStack() as p3:
          if not skip3:
            p3c = p3.enter_context(tc.tile_pool(name="p3c", bufs=1))
            lp = p3.enter_context(tc.tile_pool(name="lp", bufs=4))
            cmb = p3.enter_context(tc.tile_pool(name="cmb", bufs=4))
            tp = p3.enter_context(tc.tile_pool(name="tp", bufs=4, space="PSUM"))
            op = p3.enter_context(tc.tile_pool(name="op", bufs=4))

            id128 = p3c.tile([128, 128], f32, tag="id128")
            make_identity(nc, id128)

            for b in range(BL):
                for t in range(TC):
                    tsl = ds(t * 128, 128)
                    a0 = lp.tile([128, H], f32, tag="a0")
                    b0 = lp.tile([128, H], f32, tag="b0")
                    a1 = lp.tile([128, H], f32, tag="a1")
                    b1 = lp.tile([128, H], f32, tag="b1")
                    nc.sync.dma_start(out=a0, in_=hst[0][tsl, b, :])
                    nc.sync.dma_start(out=b0, in_=hst[1][tsl, b, :])
                    nc.sync.dma_start(out=a1, in_=hst[0][tsl, BL + b, :])
                    nc.sync.dma_start(out=b1, in_=hst[1][tsl, BL + b, :])
                    lr = cmb.tile([128, H], f32, tag="lr")
                    nc.vector.tensor_sub(lr, a0, b0)
                    li = cmb.tile([128, H], f32, tag="li")
                    nc.gpsimd.tensor_add(li, a1, b1)
                    for hb in range(2):
                        hsl = ds(hb * 128, 128)
                        ptr = tp.tile([128, 128], f32, tag="ptr")
                        nc.tensor.transpose(ptr, lr[:, hsl], id128)
                        pti = tp.tile([128, 128], f32, tag="pti")
                        nc.tensor.transpose(pti, li[:, hsl], id128)
                        ot = op.tile([128, 256], f32, tag="ot")
                        otv = ot.rearrange("p (t two) -> p t two", two=2)
                        nc.vector.tensor_copy(otv[:, :, 0], ptr)
                        nc.vector.tensor_copy(otv[:, :, 1], pti)
                        nc.sync.dma_start(
                            out=out[b, hsl, ds(2 * t * 128, 256)], in_=ot)

    nc.compile()
    return nc


_CACHE = {}
LAST_RES = None


def get_program(T):
    if T not in _CACHE:
        _CACHE[T] = build_program(T)
    return _CACHE[T]


def _pack_weights(Wih, Whh, bih, bhh):
    Wih = np.array(Wih, dtype=np.float32, copy=True)
    Whh = np.array(Whh, dtype=np.float32, copy=True)
    b = (np.asarray(bih) + np.asarray(bhh)).astype(np.float32)
    # pre-scale g gate (rows 2H:3H) by 2 so sigmoid(2g) gives tanh via 2s-1
    Wih[2 * H:3 * H] *= 2.0
    Whh[2 * H:3 * H] *= 2.0
    b[2 * H:3 * H] *= 2.0
    # permute gate blocks (i, f, g, o) -> (i, g, f, o) so the kernel's
    # zone0 = [i|g], zone32 = [f|o]
    perm = np.r_[0:H, 2 * H:3 * H, H:2 * H, 3 * H:4 * H]
    Wih = Wih[perm]
    Whh = Whh[perm]
    b = b[perm]
    return np.ascontiguousarray(Wih.T), np.ascontiguousarray(Whh.T), b


def kernel(x_real, x_imag, Wih_r, Whh_r, bih_r, bhh_r,
           Wih_i, Whh_i, bih_i, bhh_i):
    x_real = np.asarray(x_real, dtype=np.float32)
    x_imag = np.asarray(x_imag, dtype=np.float32)
    T = x_real.shape[2]
    nc = get_program(T)

    wihT_r, whhT_r, b_r = _pack_weights(Wih_r, Whh_r, bih_r, bhh_r)
    wihT_i, whhT_i, b_i = _pack_weights(Wih_i, Whh_i, bih_i, bhh_i)
    wih_p = np.ascontiguousarray(np.stack([wihT_r, wihT_i]))
    import ml_dtypes
    whh_p = np.ascontiguousarray(
        np.stack([whhT_r, whhT_i]).astype(ml_dtypes.bfloat16))
    bia_p = np.ascontiguousarray(np.stack([b_r, b_i]))
    scat_p = np.zeros((2 * R, 40), dtype=np.float32)
    for j in range(R):
        scat_p[j, j] = 1.0
        scat_p[R + j, 32 + j] = 1.0

    in_maps = []
    for c in range(NCORES):
        sl = slice(c * BL, (c + 1) * BL)
        in_maps.append({
            "xr": np.ascontiguousarray(x_real[sl]),
            "xi": np.ascontiguousarray(x_imag[sl]),
            "wih": wih_p, "whh": whh_p, "bias": bia_p,
            "scat": scat_p,
        })
    import os
    trace = os.environ.get("K_TRACE") == "1"
    res = run_bass_kernel_spmd(nc, in_maps, list(range(NCORES)), trace=trace)
    global LAST_RES
    LAST_RES = res
    parts = []
    for c in range(NCORES):
        o = np.ascontiguousarray(res.results[c]["out"])  # [BL, H, 2T] f32
        parts.append(o.view(np.complex64))               # [BL, H, T]
    return np.concatenate(parts, axis=0)


# revision 9
# speedup vs baseline: 1.6126x; 1.2310x over previous
"""ComplexLSTM Trainium2 kernel.

Problem: B=32, I=128, H=256, T=2048. Four independent LSTM scans
(real/imag weights x real/imag inputs) combined into a complex output
(B, H, T) complex64.

Sharding: data-parallel over batch across 8 cores (4 rows each); each
core runs all four scans for its batch slice, organized as two "chains"
that share a recurrent weight matrix (Whh_r / Whh_i).

Phase-2 step layout (per chain, rows R=8 = 2 slots x 4 batch):
  gates PSUM tile [40, 512], one bank: rows 0:8 = gates [i|f],
  rows 32:40 = gates [g|o] (matmul col-tile bases must be 0/32/64).
  gx(+bias) is preloaded into the bank by a scatter-identity matmul
  (K=16 -> M=40), then 4 accumulating matmuls (2 h-chunks x 2 slices)
  add h @ Whh.T. One sigmoid covers all gates (g rows pre-scaled by 2
  on host: tanh(x) = 2*sigmoid(2x)-1).
  c' = Sf*c + 2*Si*Sg - Si (GPSIMD products, DVE combine)
  h  = So * tanh(c')        (ACT + DVE)
  h is transposed on the PE (4 small transposes -> one [128,32] PSUM
  tile) and copied to SBUF with a single ACT copy as next step's
  stationary. The scatter for step k+1 is issued before step k's
  transposes so the PE has work while the elementwise tail drains.
  The real/imag combine is deferred to phase 3.
"""

import numpy as np
from contextlib import ExitStack

import concourse.bass as bass
import concourse.tile as tile
import concourse.mybir as mybir
from concourse import bacc
from concourse.bass import ds
from concourse.bass_utils import run_bass_kernel_spmd
from concourse.masks import make_identity

B, I, H = 32, 128, 256
G = 4 * H            # 1024
NCORES = 8
BL = B // NCORES     # 4 batch rows per core
R = 2 * BL           # 8 rows per chain (2 slots x 4 batch)
U = 8                # steps per For_i iteration

f32 = mybir.dt.float32
f32r = mybir.dt.float32r
bf16 = mybir.dt.bfloat16
SIG = mybir.ActivationFunctionType.Sigmoid
TANH = mybir.ActivationFunctionType.Tanh
MULT = mybir.AluOpType.mult
SUB = mybir.AluOpType.subtract


def build_program(T):
    import os
    skip1 = os.environ.get("K_SKIP1") == "1"
    skip2 = os.environ.get("K_SKIP2") == "1"
    skip3 = os.environ.get("K_SKIP3") == "1"
    TC = T // 128      # phase-1/3 tiles per (scan, b)
    nc = bacc.Bacc("TRN2", target_bir_lowering=False, debug=False,
                   num_devices=NCORES)

    xr = nc.declare_dram_parameter("xr", [BL, I, T], f32r, isOutput=False)
    xi = nc.declare_dram_parameter("xi", [BL, I, T], f32r, isOutput=False)
    wih = nc.declare_dram_parameter("wih", [2, I, G], f32r, isOutput=False)
    whh = nc.declare_dram_parameter("whh", [2, H, G], bf16, isOutput=False)
    bia = nc.declare_dram_parameter("bias", [2, G], f32, isOutput=False)
    scat = nc.declare_dram_parameter("scat", [2 * R, 40], f32r, isOutput=False)
    out = nc.declare_dram_parameter("out", [BL, H, 2 * T], f32, isOutput=True)

    # staging: gx per chain [T, 16, 512] (rows 0:8 = [i|f], 8:16 = [g|o]);
    # raw h per chain [T, R, H]
    gxst = [nc.dram_tensor(f"gx_stage{c}", [T, 2 * R, 512], f32r)
            for c in range(2)]
    hst = [nc.dram_tensor(f"h_stage{c}", [T, R, H], bf16) for c in range(2)]

    # chain c, slot s -> input tensor (chain0=Wr: xr,xi ; chain1=Wi: xi,xr)
    def xsrc(c, s):
        return (xr if s == 0 else xi) if c == 0 else (xi if s == 0 else xr)

    with tile.TileContext(nc) as tc, ExitStack() as top:
        consts = top.enter_context(tc.tile_pool(name="consts", bufs=1))

        whh_sb = [[consts.tile([128, G], bf16, name=f"whh{c}{k}",
                               tag=f"whh{c}{k}") for k in range(2)]
                  for c in range(2)]
        for c in range(2):
            for k in range(2):
                nc.sync.dma_start(out=whh_sb[c][k],
                                  in_=whh[c, k * 128:(k + 1) * 128, :])

        scat_sb = consts.tile([2 * R, 40], f32r, tag="scat_sb")
        nc.sync.dma_start(out=scat_sb, in_=scat[:, :])
        id8b = consts.tile([R, R], bf16, tag="id8b")
        make_identity(nc, id8b)

        # ---------------- phase 1: gx = x @ WihT + bias ----------------
        with ExitStack() as p1:
          if not skip1:
            p1c = p1.enter_context(tc.tile_pool(name="p1c", bufs=1))
            xp = p1.enter_context(tc.tile_pool(name="xp", bufs=4))
            gp = p1.enter_context(tc.tile_pool(name="gp", bufs=2, space="PSUM"))
            gs = p1.enter_context(tc.tile_pool(name="gs", bufs=4))

            wih_sb = [p1c.tile([I, G], f32r, name=f"wih{c}", tag=f"wih{c}")
                      for c in range(2)]
            bia_sb = [p1c.tile([128, G], f32, name=f"bia{c}", tag=f"bia{c}")
                      for c in range(2)]
            for c in range(2):
                nc.sync.dma_start(out=wih_sb[c], in_=wih[c])
                bsrc = bia[c:c + 1, :]
                nc.sync.dma_start(
                    out=bia_sb[c],
                    in_=bass.AP(tensor=bsrc.tensor, offset=bsrc.offset,
                                ap=[[0, 128]] + list(bsrc.ap[-1:])))

            for c in range(2):
                for s in range(2):
                    src = xsrc(c, s)
                    for b in range(BL):
                        row = s * BL + b
                        for t in range(TC):
                            xt = xp.tile([I, 128], f32r, tag="xt")
                            nc.sync.dma_start(
                                out=xt, in_=src[b, :, t * 128:(t + 1) * 128])
                            ps = gp.tile([128, G], f32, tag="ps")
                            for n in range(2):
                                sl = ds(n * 512, 512)
                                nc.tensor.matmul(ps[:, sl], xt,
                                                 wih_sb[c][:, sl],
                                                 start=True, stop=True)
                            gt = gs.tile([128, G], f32r, tag="gt")
                            nc.vector.tensor_add(gt, ps, bia_sb[c])
                            tsl = ds(t * 128, 128)
                            nc.sync.dma_start(
                                out=gxst[c][tsl, row, :], in_=gt[:, 0:512])
                            nc.sync.dma_start(
                                out=gxst[c][tsl, R + row, :],
                                in_=gt[:, 512:1024])

        # ---------------- phase 2: the recurrence ----------------
        with ExitStack() as p2:
          if not skip2:
            st8 = p2.enter_context(tc.tile_pool(name="st8", bufs=1))
            gxp = p2.enter_context(tc.tile_pool(name="gxp", bufs=2))
            spl = p2.enter_context(tc.tile_pool(name="spl", bufs=2))
            tmp = p2.enter_context(tc.tile_pool(name="tmp", bufs=2))
            stg = p2.enter_context(tc.tile_pool(name="stg", bufs=2))
            psA = p2.enter_context(tc.tile_pool(name="psA", bufs=2, space="PSUM"))
            psB = p2.enter_context(tc.tile_pool(name="psB", bufs=2, space="PSUM"))
            psT = p2.enter_context(tc.tile_pool(name="psT", bufs=2, space="PSUM"))

            # persistent state (ping-pong on step parity), split per chain
            # so the two chains' dependency cycles stay independent.
            # hTc[c][p]: [128, 16] cols hb*8..+8 = h[c]^T chunk hb
            # c state lives at base partition 32 (rows 32:40) to match the
            # [f|o] gate zone
            hTc = [[st8.tile([128, 2 * R], bf16, name=f"hT{c}{p}",
                             tag=f"hT{c}{p}") for p in range(2)]
                   for c in range(2)]
            cst = [[st8.tile([40, H], f32, name=f"c{c}{p}", tag=f"c{c}{p}")
                    for p in range(2)] for c in range(2)]
            for p in range(2):
                for c in range(2):
                    nc.vector.memset(hTc[c][p], 0.0)
                    nc.vector.memset(cst[c][p], 0.0)

            stt = nc.vector.scalar_tensor_tensor

            def hT_sl(p, c, hb):
                return hTc[c][p][:, hb * R:(hb + 1) * R]

            with tc.For_i(0, T, U, staggered_reset=True) as iv:
                gxch = [gxp.tile([2 * R, U, 512], f32r, name=f"gx{c}",
                                 tag=f"gx{c}") for c in range(2)]
                for c in range(2):
                    nc.sync.dma_start(
                        out=gxch[c],
                        in_=gxst[c][ds(iv, U), :, :].rearrange(
                            "u p g -> p u g"))
                st = [stg.tile([R, U, H], bf16, name=f"st{c}", tag=f"st{c}")
                      for c in range(2)]

                def scatter(c, kk):
                    pool = psA if c == 0 else psB
                    g_ = pool.tile([40, 512], f32, name=f"G{c}", tag=f"G{c}")
                    nc.tensor.matmul(g_, scat_sb, gxch[c][:, kk, :],
                                     start=True, stop=True)
                    return g_

                Gcur = None
                for k in range(U):
                    pp = k % 2
                    if k == 0:
                        Gcur = [scatter(c, 0) for c in range(2)]
                    # gate matmuls accumulate onto gx
                    for c in range(2):
                        for s in range(2):
                            for kc in range(2):
                                nc.tensor.matmul(
                                    Gcur[c][s * 32:s * 32 + R, :],
                                    hT_sl(pp, c, kc),
                                    whh_sb[c][kc][:, s * 512:(s + 1) * 512],
                                    start=False, stop=True,
                                    skip_group_check=True)
                    # prefetch next step's gx into fresh psum banks
                    Gnext = ([scatter(c, k + 1) for c in range(2)]
                             if k + 1 < U else None)
                    # sigmoids (gate cols permuted on host to [i g | f o]:
                    # zone0 rows 0:8 = i|g, zone32 rows 32:40 = f|o)
                    S, slc = [], {}
                    for c in range(2):
                        S_ = spl.tile([40, 512], f32, name=f"S{c}",
                                      tag=f"S{c}")
                        nc.scalar.activation(S_, Gcur[c], SIG)
                        S.append(S_)
                        slc[c] = (S_[0:R, 0:256], S_[32:32 + R, 0:256],
                                  S_[0:R, 256:512], S_[32:32 + R, 256:512])
                    # c update: GPSIMD takes p (the one product with no
                    # DVE-chain dependency); DVE runs each chain's
                    # dependent tail consecutively so chain A's h is not
                    # queued behind chain B's ops
                    pt_ = {}
                    for c in range(2):
                        Si, Sf, Sg, So = slc[c]
                        p_ = tmp.tile([R, H], f32, name=f"p{c}", tag=f"p{c}")
                        nc.gpsimd.tensor_mul(p_, Si, Sg)
                        pt_[c] = p_
                    for c in range(2):
                        Si, Sf, Sg, So = slc[c]
                        v_ = tmp.tile([R, H], f32, name=f"v{c}", tag=f"v{c}")
                        nc.vector.tensor_mul(v_, Sf, cst[c][pp][32:40, :])
                        u_ = tmp.tile([R, H], f32, name=f"u{c}", tag=f"u{c}")
                        stt(out=u_, in0=pt_[c], scalar=2.0, in1=Si,
                            op0=MULT, op1=SUB)
                        cn = cst[c][1 - pp][32:40, :]
                        nc.vector.tensor_add(cn, u_, v_)
                        tc_ = tmp.tile([40, H], f32, name=f"tc{c}",
                                       tag=f"tc{c}")
                        nc.scalar.activation(tc_[32:40, :], cn, TANH)
                        # h = sig(o) * tanh(c) -> store buffer (bf16)
                        nc.vector.tensor_mul(st[c][:, k, :], So,
                                             tc_[32:40, :])
                        ptt = psT.tile([128, 2 * R], bf16, name=f"ptt{c}",
                                       tag=f"ptt{c}")
                        for hb in range(2):
                            nc.tensor.transpose(
                                ptt[:, hb * R:(hb + 1) * R],
                                st[c][:, k, hb * 128:(hb + 1) * 128], id8b)
                        nc.scalar.copy(hTc[c][1 - pp], ptt)
                    Gcur = Gnext
                for c in range(2):
                    nc.gpsimd.dma_start(
                        out=hst[c][ds(iv, U), :, :].rearrange("u p h -> p u h"),
                        in_=st[c])

        # ------- phase 3: combine, transpose to (b, h, t), interleave -----
        with ExitStack() as p3:
          if not skip3:
            p3c = p3.enter_context(tc.tile_pool(name="p3c", bufs=1))
            lp = p3.enter_context(tc.tile_pool(name="lp", bufs=4))
            cmb = p3.enter_context(tc.tile_pool(name="cmb", bufs=4))
            tp = p3.enter_context(tc.tile_pool(name="tp", bufs=4, space="PSUM"))
            op = p3.enter_context(tc.tile_pool(name="op", bufs=4))

            id128 = p3c.tile([128, 128], f32, tag="id128")
            make_identity(nc, id128)

            for b in range(BL):
                for t in range(TC):
                    tsl = ds(t * 128, 128)
                    a0 = lp.tile([128, H], bf16, tag="a0")
                    b0 = lp.tile([128, H], bf16, tag="b0")
                    a1 = lp.tile([128, H], bf16, tag="a1")
                    b1 = lp.tile([128, H], bf16, tag="b1")
                    nc.sync.dma_start(out=a0, in_=hst[0][tsl, b, :])
                    nc.sync.dma_start(out=b0, in_=hst[1][tsl, b, :])
                    nc.sync.dma_start(out=a1, in_=hst[0][tsl, BL + b, :])
                    nc.sync.dma_start(out=b1, in_=hst[1][tsl, BL + b, :])
                    lr = cmb.tile([128, H], f32, tag="lr")
                    nc.vector.tensor_sub(lr, a0, b0)
                    li = cmb.tile([128, H], f32, tag="li")
                    nc.gpsimd.tensor_add(li, a1, b1)
                    for hb in range(2):
                        hsl = ds(hb * 128, 128)
                        ptr = tp.tile([128, 128], f32, tag="ptr")
                        nc.tensor.transpose(ptr, lr[:, hsl], id128)
                        pti = tp.tile([128, 128], f32, tag="pti")
                        nc.tensor.transpose(pti, li[:, hsl], id128)
                        ot = op.tile([128, 256], f32, tag="ot")
                        otv = ot.rearrange("p (t two) -> p t two", two=2)
                        nc.vector.tensor_copy(otv[:, :, 0], ptr)
                        nc.vector.tensor_copy(otv[:, :, 1], pti)
                        nc.sync.dma_start(
                            out=out[b, hsl, ds(2 * t * 128, 256)], in_=ot)

    nc.compile()
    return nc


_CACHE = {}
LAST_RES = None


def get_program(T):
    if T not in _CACHE:
        _CACHE[T] = build_program(T)
    return _CACHE[T]


def _pack_weights(Wih, Whh, bih, bhh):
    Wih = np.array(Wih, dtype=np.float32, copy=True)
    Whh = np.array(Whh, dtype=np.float32, copy=True)
    b = (np.asarray(bih) + np.asarray(bhh)).astype(np.float32)
    # pre-scale g gate (rows 2H:3H) by 2 so sigmoid(2g) gives tanh via 2s-1
    Wih[2 * H:3 * H] *= 2.0
    Whh[2 * H:3 * H] *= 2.0
    b[2 * H:3 * H] *= 2.0
    # permute gate blocks (i, f, g, o) -> (i, g, f, o) so the kernel's
    # zone0 = [i|g], zone32 = [f|o]
    perm = np.r_[0:H, 2 * H:3 * H, H:2 * H, 3 * H:4 * H]
    Wih = Wih[perm]
    Whh = Whh[perm]
    b = b[perm]
    return np.ascontiguousarray(Wih.T), np.ascontiguousarray(Whh.T), b


def kernel(x_real, x_imag, Wih_r, Whh_r, bih_r, bhh_r,
           Wih_i, Whh_i, bih_i, bhh_i):
    x_real = np.asarray(x_real, dtype=np.float32)
    x_imag = np.asarray(x_imag, dtype=np.float32)
    T = x_real.shape[2]
    nc = get_program(T)

    wihT_r, whhT_r, b_r = _pack_weights(Wih_r, Whh_r, bih_r, bhh_r)
    wihT_i, whhT_i, b_i = _pack_weights(Wih_i, Whh_i, bih_i, bhh_i)
    wih_p = np.ascontiguousarray(np.stack([wihT_r, wihT_i]))
    import ml_dtypes
    whh_p = np.ascontiguousarray(
        np.stack([whhT_r, whhT_i]).astype(ml_dtypes.bfloat16))
    bia_p = np.ascontiguousarray(np.stack([b_r, b_i]))
    scat_p = np.zeros((2 * R, 40), dtype=np.float32)
    for j in range(R):
        scat_p[j, j] = 1.0
        scat_p[R + j, 32 + j] = 1.0

    in_maps = []
    for c in range(NCORES):
        sl = slice(c * BL, (c + 1) * BL)
        in_maps.append({
            "xr": np.ascontiguousarray(x_real[sl]),
            "xi": np.ascontiguousarray(x_imag[sl]),
            "wih": wih_p, "whh": whh_p, "bias": bia_p,
            "scat": scat_p,
        })
    import os
    trace = os.environ.get("K_TRACE") == "1"
    res = run_bass_kernel_spmd(nc, in_maps, list(range(NCORES)), trace=trace)
    global LAST_RES
    LAST_RES = res
    parts = []
    for c in range(NCORES):
        o = np.ascontiguousarray(res.results[c]["out"])  # [BL, H, 2T] f32
        parts.append(o.view(np.complex64))               # [BL, H, T]
    return np.concatenate(parts, axis=0)


# revision 10
# speedup vs baseline: 1.7021x; 1.0555x over previous
"""ComplexLSTM Trainium2 kernel.

Problem: B=32, I=128, H=256, T=2048. Four independent LSTM scans
(real/imag weights x real/imag inputs) combined into a complex output
(B, H, T) complex64.

Sharding: data-parallel over batch across 8 cores (4 rows each); each
core runs all four scans for its batch slice, organized as two "chains"
that share a recurrent weight matrix (Whh_r / Whh_i).

Phase-2 step layout (per chain, rows R=8 = 2 slots x 4 batch):
  gates PSUM tile [40, 512], one bank: rows 0:8 = gates [i|f],
  rows 32:40 = gates [g|o] (matmul col-tile bases must be 0/32/64).
  gx(+bias) is preloaded into the bank by a scatter-identity matmul
  (K=16 -> M=40), then 4 accumulating matmuls (2 h-chunks x 2 slices)
  add h @ Whh.T. One sigmoid covers all gates (g rows pre-scaled by 2
  on host: tanh(x) = 2*sigmoid(2x)-1).
  c' = Sf*c + 2*Si*Sg - Si (GPSIMD products, DVE combine)
  h  = So * tanh(c')        (ACT + DVE)
  h is transposed on the PE (4 small transposes -> one [128,32] PSUM
  tile) and copied to SBUF with a single ACT copy as next step's
  stationary. The scatter for step k+1 is issued before step k's
  transposes so the PE has work while the elementwise tail drains.
  The real/imag combine is deferred to phase 3.
"""

import numpy as np
from contextlib import ExitStack

import concourse.bass as bass
import concourse.tile as tile
import concourse.mybir as mybir
from concourse import bacc
from concourse.bass import ds
from concourse.bass_utils import run_bass_kernel_spmd
from concourse.masks import make_identity

B, I, H = 32, 128, 256
G = 4 * H            # 1024
NCORES = 8
BL = B // NCORES     # 4 batch rows per core
R = 2 * BL           # 8 rows per chain (2 slots x 4 batch)
U = 8                # steps per For_i iteration

f32 = mybir.dt.float32
f32r = mybir.dt.float32r
bf16 = mybir.dt.bfloat16
SIG = mybir.ActivationFunctionType.Sigmoid
TANH = mybir.ActivationFunctionType.Tanh
MULT = mybir.AluOpType.mult
SUB = mybir.AluOpType.subtract


def build_program(T):
    import os
    skip1 = os.environ.get("K_SKIP1") == "1"
    skip2 = os.environ.get("K_SKIP2") == "1"
    skip3 = os.environ.get("K_SKIP3") == "1"
    TC = T // 128      # phase-1/3 tiles per (scan, b)
    nc = bacc.Bacc("TRN2", target_bir_lowering=False, debug=False,
                   num_devices=NCORES)

    xr = nc.declare_dram_parameter("xr", [BL, I, T], f32r, isOutput=False)
    xi = nc.declare_dram_parameter("xi", [BL, I, T], f32r, isOutput=False)
    wih = nc.declare_dram_parameter("wih", [2, I, G], f32r, isOutput=False)
    whh = nc.declare_dram_parameter("whh", [2, H, G], bf16, isOutput=False)
    bia = nc.declare_dram_parameter("bias", [2, G], f32, isOutput=False)
    scat = nc.declare_dram_parameter("scat", [2 * R, 40], f32r, isOutput=False)
    out = nc.declare_dram_parameter("out", [BL, H, 2 * T], f32, isOutput=True)

    # staging: gx per chain [T, 16, 512] (rows 0:8 = [i|f], 8:16 = [g|o]);
    # raw h per chain [T, R, H]
    gxst = [nc.dram_tensor(f"gx_stage{c}", [T, 2 * R, 512], f32r)
            for c in range(2)]
    hst = [nc.dram_tensor(f"h_stage{c}", [T, R, H], bf16) for c in range(2)]

    # chain c, slot s -> input tensor (chain0=Wr: xr,xi ; chain1=Wi: xi,xr)
    def xsrc(c, s):
        return (xr if s == 0 else xi) if c == 0 else (xi if s == 0 else xr)

    with tile.TileContext(nc) as tc, ExitStack() as top:
        consts = top.enter_context(tc.tile_pool(name="consts", bufs=1))

        whh_sb = [[consts.tile([128, G], bf16, name=f"whh{c}{k}",
                               tag=f"whh{c}{k}") for k in range(2)]
                  for c in range(2)]
        for c in range(2):
            for k in range(2):
                nc.sync.dma_start(out=whh_sb[c][k],
                                  in_=whh[c, k * 128:(k + 1) * 128, :])

        scat_sb = consts.tile([2 * R, 40], f32r, tag="scat_sb")
        nc.sync.dma_start(out=scat_sb, in_=scat[:, :])
        id8b = consts.tile([R, R], bf16, tag="id8b")
        make_identity(nc, id8b)

        # ---------------- phase 1: gx = x @ WihT + bias ----------------
        with ExitStack() as p1:
          if not skip1:
            p1c = p1.enter_context(tc.tile_pool(name="p1c", bufs=1))
            xp = p1.enter_context(tc.tile_pool(name="xp", bufs=4))
            gp = p1.enter_context(tc.tile_pool(name="gp", bufs=2, space="PSUM"))
            gs = p1.enter_context(tc.tile_pool(name="gs", bufs=4))

            wih_sb = [p1c.tile([I, G], f32r, name=f"wih{c}", tag=f"wih{c}")
                      for c in range(2)]
            bia_sb = [p1c.tile([128, G], f32, name=f"bia{c}", tag=f"bia{c}")
                      for c in range(2)]
            for c in range(2):
                nc.sync.dma_start(out=wih_sb[c], in_=wih[c])
                bsrc = bia[c:c + 1, :]
                nc.sync.dma_start(
                    out=bia_sb[c],
                    in_=bass.AP(tensor=bsrc.tensor, offset=bsrc.offset,
                                ap=[[0, 128]] + list(bsrc.ap[-1:])))

            for c in range(2):
                for s in range(2):
                    src = xsrc(c, s)
                    for b in range(BL):
                        row = s * BL + b
                        for t in range(TC):
                            xt = xp.tile([I, 128], f32r, tag="xt")
                            nc.sync.dma_start(
                                out=xt, in_=src[b, :, t * 128:(t + 1) * 128])
                            ps = gp.tile([128, G], f32, tag="ps")
                            for n in range(2):
                                sl = ds(n * 512, 512)
                                nc.tensor.matmul(ps[:, sl], xt,
                                                 wih_sb[c][:, sl],
                                                 start=True, stop=True)
                            gt = gs.tile([128, G], f32r, tag="gt")
                            nc.vector.tensor_add(gt, ps, bia_sb[c])
                            tsl = ds(t * 128, 128)
                            nc.sync.dma_start(
                                out=gxst[c][tsl, row, :], in_=gt[:, 0:512])
                            nc.sync.dma_start(
                                out=gxst[c][tsl, R + row, :],
                                in_=gt[:, 512:1024])

        # ---------------- phase 2: the recurrence ----------------
        with ExitStack() as p2:
          if not skip2:
            st8 = p2.enter_context(tc.tile_pool(name="st8", bufs=1))
            gxp = p2.enter_context(tc.tile_pool(name="gxp", bufs=2))
            spl = p2.enter_context(tc.tile_pool(name="spl", bufs=2))
            tmp = p2.enter_context(tc.tile_pool(name="tmp", bufs=2))
            stg = p2.enter_context(tc.tile_pool(name="stg", bufs=2))
            psA = p2.enter_context(tc.tile_pool(name="psA", bufs=3, space="PSUM"))
            psB = p2.enter_context(tc.tile_pool(name="psB", bufs=3, space="PSUM"))
            psT = p2.enter_context(tc.tile_pool(name="psT", bufs=1, space="PSUM"))

            # persistent state (ping-pong on step parity), split per chain
            # so the two chains' dependency cycles stay independent.
            # hTc[c][p]: [128, 16] cols hb*8..+8 = h[c]^T chunk hb
            # c state lives at base partition 32 (rows 32:40) to match the
            # [f|o] gate zone
            hTc = [[st8.tile([128, 2 * R], bf16, name=f"hT{c}{p}",
                             tag=f"hT{c}{p}") for p in range(2)]
                   for c in range(2)]
            cst = [[st8.tile([40, H], f32, name=f"c{c}{p}", tag=f"c{c}{p}")
                    for p in range(2)] for c in range(2)]
            for p in range(2):
                for c in range(2):
                    nc.vector.memset(hTc[c][p], 0.0)
                    nc.vector.memset(cst[c][p], 0.0)

            stt = nc.vector.scalar_tensor_tensor

            def hT_sl(p, c, hb):
                return hTc[c][p][:, hb * R:(hb + 1) * R]

            with tc.For_i(0, T, U, staggered_reset=True) as iv:
                gxch = [gxp.tile([2 * R, U, 512], f32r, name=f"gx{c}",
                                 tag=f"gx{c}") for c in range(2)]
                for c in range(2):
                    nc.sync.dma_start(
                        out=gxch[c],
                        in_=gxst[c][ds(iv, U), :, :].rearrange(
                            "u p g -> p u g"))
                st = [stg.tile([R, U, H], bf16, name=f"st{c}", tag=f"st{c}")
                      for c in range(2)]

                def scatter(c, kk):
                    pool = psA if c == 0 else psB
                    g_ = pool.tile([40, 512], f32, name=f"G{c}", tag=f"G{c}")
                    nc.tensor.matmul(g_, scat_sb, gxch[c][:, kk, :],
                                     start=True, stop=True)
                    return g_

                Gcur = None
                for k in range(U):
                    pp = k % 2
                    if k == 0:
                        Gcur = [scatter(c, 0) for c in range(2)]
                    # gate matmuls accumulate onto gx (h-chunk outer so
                    # the first two only wait on hT chunk 0)
                    for c in range(2):
                        for kc in range(2):
                            for s in range(2):
                                nc.tensor.matmul(
                                    Gcur[c][s * 32:s * 32 + R, :],
                                    hT_sl(pp, c, kc),
                                    whh_sb[c][kc][:, s * 512:(s + 1) * 512],
                                    start=False, stop=True,
                                    skip_group_check=True)
                    # prefetch next step's gx into fresh psum banks
                    Gnext = ([scatter(c, k + 1) for c in range(2)]
                             if k + 1 < U else None)
                    # sigmoids (gate cols permuted on host to [i g | f o]:
                    # zone0 rows 0:8 = i|g, zone32 rows 32:40 = f|o)
                    S, slc = [], {}
                    for c in range(2):
                        S_ = spl.tile([40, 512], f32, name=f"S{c}",
                                      tag=f"S{c}")
                        nc.scalar.activation(S_, Gcur[c], SIG)
                        S.append(S_)
                        slc[c] = (S_[0:R, 0:256], S_[32:32 + R, 0:256],
                                  S_[0:R, 256:512], S_[32:32 + R, 256:512])
                    # c update: GPSIMD takes v = Sf*c (off the critical
                    # cycle); DVE runs each chain's dependent tail
                    # [p, u, cn, h] consecutively so chain A's tail is not
                    # queued behind chain B's ops
                    for c in range(2):
                        Sf = slc[c][1]
                        v_ = tmp.tile([R, H], f32, name=f"v{c}", tag=f"v{c}")
                        nc.gpsimd.tensor_mul(v_, Sf, cst[c][pp][32:40, :])
                        slc[c] = slc[c] + (v_,)
                    for c in range(2):
                        Si, Sf, Sg, So, v_ = slc[c]
                        p_ = tmp.tile([R, H], f32, name=f"p{c}", tag=f"p{c}")
                        nc.vector.tensor_mul(p_, Si, Sg)
                        u_ = tmp.tile([R, H], f32, name=f"u{c}", tag=f"u{c}")
                        stt(out=u_, in0=p_, scalar=2.0, in1=Si,
                            op0=MULT, op1=SUB)
                        cn = cst[c][1 - pp][32:40, :]
                        nc.vector.tensor_add(cn, u_, v_)
                        tc_ = tmp.tile([40, H], f32, name=f"tc{c}",
                                       tag=f"tc{c}")
                        nc.scalar.activation(tc_[32:40, :], cn, TANH)
                        # h = sig(o) * tanh(c) -> store buffer (bf16)
                        nc.vector.tensor_mul(st[c][:, k, :], So,
                                             tc_[32:40, :])
                        ptt = psT.tile([128, 2 * R], bf16, name=f"ptt{c}",
                                       tag=f"ptt{c}")
                        # per-chunk transpose+copy: the next step's hb=0
                        # matmuls only wait for chunk 0
                        nc.tensor.transpose(ptt[:, 0:R],
                                            st[c][:, k, 0:128], id8b)
                        nc.scalar.copy(hTc[c][1 - pp][:, 0:R], ptt[:, 0:R])
                        nc.tensor.transpose(ptt[:, R:2 * R],
                                            st[c][:, k, 128:256], id8b)
                        nc.vector.tensor_copy(hTc[c][1 - pp][:, R:2 * R],
                                              ptt[:, R:2 * R])
                    Gcur = Gnext
                for c in range(2):
                    nc.gpsimd.dma_start(
                        out=hst[c][ds(iv, U), :, :].rearrange("u p h -> p u h"),
                        in_=st[c])

        # ------- phase 3: combine, transpose to (b, h, t), interleave -----
        with ExitStack() as p3:
          if not skip3:
            p3c = p3.enter_context(tc.tile_pool(name="p3c", bufs=1))
            lp = p3.enter_context(tc.tile_pool(name="lp", bufs=4))
            cmb = p3.enter_context(tc.tile_pool(name="cmb", bufs=4))
            tp = p3.enter_context(tc.tile_pool(name="tp", bufs=4, space="PSUM"))
            op = p3.enter_context(tc.tile_pool(name="op", bufs=4))

            id128 = p3c.tile([128, 128], f32, tag="id128")
            make_identity(nc, id128)

            for b in range(BL):
                for t in range(TC):
                    tsl = ds(t * 128, 128)
                    a0 = lp.tile([128, H], bf16, tag="a0")
                    b0 = lp.tile([128, H], bf16, tag="b0")
                    a1 = lp.tile([128, H], bf16, tag="a1")
                    b1 = lp.tile([128, H], bf16, tag="b1")
                    nc.sync.dma_start(out=a0, in_=hst[0][tsl, b, :])
                    nc.sync.dma_start(out=b0, in_=hst[1][tsl, b, :])
                    nc.sync.dma_start(out=a1, in_=hst[0][tsl, BL + b, :])
                    nc.sync.dma_start(out=b1, in_=hst[1][tsl, BL + b, :])
                    lr = cmb.tile([128, H], f32, tag="lr")
                    nc.vector.tensor_sub(lr, a0, b0)
                    li = cmb.tile([128, H], f32, tag="li")
                    nc.gpsimd.tensor_add(li, a1, b1)
                    for hb in range(2):
                        hsl = ds(hb * 128, 128)
                        ptr = tp.tile([128, 128], f32, tag="ptr")
                        nc.tensor.transpose(ptr, lr[:, hsl], id128)
                        pti = tp.tile([128, 128], f32, tag="pti")
                        nc.tensor.transpose(pti, li[:, hsl], id128)
                        ot = op.tile([128, 256], f32, tag="ot")
                        otv = ot.rearrange("p (t two) -> p t two", two=2)
                        nc.vector.tensor_copy(otv[:, :, 0], ptr)
                        nc.vector.tensor_copy(otv[:, :, 1], pti)
                        nc.sync.dma_start(
                            out=out[b, hsl, ds(2 * t * 128, 256)], in_=ot)

    nc.compile()
    return nc


_CACHE = {}
LAST_RES = None


def get_program(T):
    if T not in _CACHE:
        _CACHE[T] = build_program(T)
    return _CACHE[T]


def _pack_weights(Wih, Whh, bih, bhh):
    Wih = np.array(Wih, dtype=np.float32, copy=True)
    Whh = np.array(Whh, dtype=np.float32, copy=True)
    b = (np.asarray(bih) + np.asarray(bhh)).astype(np.float32)
    # pre-scale g gate (rows 2H:3H) by 2 so sigmoid(2g) gives tanh via 2s-1
    Wih[2 * H:3 * H] *= 2.0
    Whh[2 * H:3 * H] *= 2.0
    b[2 * H:3 * H] *= 2.0
    # permute gate blocks (i, f, g, o) -> (i, g, f, o) so the kernel's
    # zone0 = [i|g], zone32 = [f|o]
    perm = np.r_[0:H, 2 * H:3 * H, H:2 * H, 3 * H:4 * H]
    Wih = Wih[perm]
    Whh = Whh[perm]
    b = b[perm]
    return np.ascontiguousarray(Wih.T), np.ascontiguousarray(Whh.T), b


def kernel(x_real, x_imag, Wih_r, Whh_r, bih_r, bhh_r,
           Wih_i, Whh_i, bih_i, bhh_i):
    x_real = np.asarray(x_real, dtype=np.float32)
    x_imag = np.asarray(x_imag, dtype=np.float32)
    T = x_real.shape[2]
    nc = get_program(T)

    wihT_r, whhT_r, b_r = _pack_weights(Wih_r, Whh_r, bih_r, bhh_r)
    wihT_i, whhT_i, b_i = _pack_weights(Wih_i, Whh_i, bih_i, bhh_i)
    wih_p = np.ascontiguousarray(np.stack([wihT_r, wihT_i]))
    import ml_dtypes
    whh_p = np.ascontiguousarray(
        np.stack([whhT_r, whhT_i]).astype(ml_dtypes.bfloat16))
    bia_p = np.ascontiguousarray(np.stack([b_r, b_i]))
    scat_p = np.zeros((2 * R, 40), dtype=np.float32)
    for j in range(R):
        scat_p[j, j] = 1.0
        scat_p[R + j, 32 + j] = 1.0

    in_maps = []
    for c in range(NCORES):
        sl = slice(c * BL, (c + 1) * BL)
        in_maps.append({
            "xr": np.ascontiguousarray(x_real[sl]),
            "xi": np.ascontiguousarray(x_imag[sl]),
            "wih": wih_p, "whh": whh_p, "bias": bia_p,
            "scat": scat_p,
        })
    import os
    trace = os.environ.get("K_TRACE") == "1"
    res = run_bass_kernel_spmd(nc, in_maps, list(range(NCORES)), trace=trace)
    global LAST_RES
    LAST_RES = res
    parts = []
    for c in range(NCORES):
        o = np.ascontiguousarray(res.results[c]["out"])  # [BL, H, 2T] f32
        parts.append(o.view(np.complex64))               # [BL, H, T]
    return np.concatenate(parts, axis=0)


# revision 11
# speedup vs baseline: 1.7869x; 1.0498x over previous
"""ComplexLSTM Trainium2 kernel.

Problem: B=32, I=128, H=256, T=2048. Four independent LSTM scans
(real/imag weights x real/imag inputs) combined into a complex output
(B, H, T) complex64.

Sharding: data-parallel over batch across 8 cores (4 rows each); each
core runs all four scans for its batch slice, organized as two "chains"
that share a recurrent weight matrix (Whh_r / Whh_i).

Phase-2 step layout (per chain, rows R=8 = 2 slots x 4 batch):
  gates PSUM tile [40, 512], one bank: rows 0:8 = gates [i|f],
  rows 32:40 = gates [g|o] (matmul col-tile bases must be 0/32/64).
  gx(+bias) is preloaded into the bank by a scatter-identity matmul
  (K=16 -> M=40), then 4 accumulating matmuls (2 h-chunks x 2 slices)
  add h @ Whh.T. One sigmoid covers all gates (g rows pre-scaled by 2
  on host: tanh(x) = 2*sigmoid(2x)-1).
  c' = Sf*c + 2*Si*Sg - Si (GPSIMD products, DVE combine)
  h  = So * tanh(c')        (ACT + DVE)
  h is transposed on the PE (4 small transposes -> one [128,32] PSUM
  tile) and copied to SBUF with a single ACT copy as next step's
  stationary. The scatter for step k+1 is issued before step k's
  transposes so the PE has work while the elementwise tail drains.
  The real/imag combine is deferred to phase 3.
"""

import numpy as np
from contextlib import ExitStack

import concourse.bass as bass
import concourse.tile as tile
import concourse.mybir as mybir
from concourse import bacc
from concourse.bass import ds
from concourse.bass_utils import run_bass_kernel_spmd
from concourse.masks import make_identity

B, I, H = 32, 128, 256
G = 4 * H            # 1024
NCORES = 8
BL = B // NCORES     # 4 batch rows per core
R = 2 * BL           # 8 rows per chain (2 slots x 4 batch)
U = 8                # steps per For_i iteration

f32 = mybir.dt.float32
f32r = mybir.dt.float32r
bf16 = mybir.dt.bfloat16
SIG = mybir.ActivationFunctionType.Sigmoid
TANH = mybir.ActivationFunctionType.Tanh
MULT = mybir.AluOpType.mult
SUB = mybir.AluOpType.subtract


def build_program(T):
    import os
    skip1 = os.environ.get("K_SKIP1") == "1"
    skip2 = os.environ.get("K_SKIP2") == "1"
    skip3 = os.environ.get("K_SKIP3") == "1"
    TC = T // 128      # phase-1/3 tiles per (scan, b)
    nc = bacc.Bacc("TRN2", target_bir_lowering=False, debug=False,
                   num_devices=NCORES)

    xr = nc.declare_dram_parameter("xr", [BL, I, T], f32r, isOutput=False)
    xi = nc.declare_dram_parameter("xi", [BL, I, T], f32r, isOutput=False)
    wih = nc.declare_dram_parameter("wih", [2, I, G], f32r, isOutput=False)
    whh = nc.declare_dram_parameter("whh", [2, H, G], bf16, isOutput=False)
    bia = nc.declare_dram_parameter("bias", [2, G], f32, isOutput=False)
    scat = nc.declare_dram_parameter("scat", [2 * R, 40], f32r, isOutput=False)
    out = nc.declare_dram_parameter("out", [BL, H, 2 * T], f32, isOutput=True)

    # staging: gx per chain [T, 16, 512] (rows 0:8 = [i|f], 8:16 = [g|o]);
    # raw h per chain [T, R, H]
    gxst = [nc.dram_tensor(f"gx_stage{c}", [T, 2 * R, 512], f32r)
            for c in range(2)]
    hst = [nc.dram_tensor(f"h_stage{c}", [T, R, H], bf16) for c in range(2)]

    # chain c, slot s -> input tensor (chain0=Wr: xr,xi ; chain1=Wi: xi,xr)
    def xsrc(c, s):
        return (xr if s == 0 else xi) if c == 0 else (xi if s == 0 else xr)

    with tile.TileContext(nc) as tc, ExitStack() as top:
        consts = top.enter_context(tc.tile_pool(name="consts", bufs=1))

        whh_sb = [[consts.tile([128, G], bf16, name=f"whh{c}{k}",
                               tag=f"whh{c}{k}") for k in range(2)]
                  for c in range(2)]
        for c in range(2):
            for k in range(2):
                nc.sync.dma_start(out=whh_sb[c][k],
                                  in_=whh[c, k * 128:(k + 1) * 128, :])

        scat_sb = consts.tile([2 * R, 40], f32r, tag="scat_sb")
        nc.sync.dma_start(out=scat_sb, in_=scat[:, :])
        id8b = consts.tile([R, R], bf16, tag="id8b")
        make_identity(nc, id8b)

        # ---------------- phase 1: gx = x @ WihT + bias ----------------
        with ExitStack() as p1:
          if not skip1:
            p1c = p1.enter_context(tc.tile_pool(name="p1c", bufs=1))
            xp = p1.enter_context(tc.tile_pool(name="xp", bufs=4))
            gp = p1.enter_context(tc.tile_pool(name="gp", bufs=2, space="PSUM"))
            gs = p1.enter_context(tc.tile_pool(name="gs", bufs=4))

            wih_sb = [p1c.tile([I, G], f32r, name=f"wih{c}", tag=f"wih{c}")
                      for c in range(2)]
            bia_sb = [p1c.tile([128, G], f32, name=f"bia{c}", tag=f"bia{c}")
                      for c in range(2)]
            for c in range(2):
                nc.sync.dma_start(out=wih_sb[c], in_=wih[c])
                bsrc = bia[c:c + 1, :]
                nc.sync.dma_start(
                    out=bia_sb[c],
                    in_=bass.AP(tensor=bsrc.tensor, offset=bsrc.offset,
                                ap=[[0, 128]] + list(bsrc.ap[-1:])))

            for c in range(2):
                for s in range(2):
                    src = xsrc(c, s)
                    for b in range(BL):
                        row = s * BL + b
                        for t in range(TC):
                            xt = xp.tile([I, 128], f32r, tag="xt")
                            nc.sync.dma_start(
                                out=xt, in_=src[b, :, t * 128:(t + 1) * 128])
                            ps = gp.tile([128, G], f32, tag="ps")
                            for n in range(2):
                                sl = ds(n * 512, 512)
                                nc.tensor.matmul(ps[:, sl], xt,
                                                 wih_sb[c][:, sl],
                                                 start=True, stop=True)
                            gt = gs.tile([128, G], f32r, tag="gt")
                            nc.vector.tensor_add(gt, ps, bia_sb[c])
                            tsl = ds(t * 128, 128)
                            nc.sync.dma_start(
                                out=gxst[c][tsl, row, :], in_=gt[:, 0:512])
                            nc.sync.dma_start(
                                out=gxst[c][tsl, R + row, :],
                                in_=gt[:, 512:1024])

        # ---------------- phase 2: the recurrence ----------------
        with ExitStack() as p2:
          if not skip2:
            st8 = p2.enter_context(tc.tile_pool(name="st8", bufs=1))
            gxp = p2.enter_context(tc.tile_pool(name="gxp", bufs=2))
            spl = p2.enter_context(tc.tile_pool(name="spl", bufs=2))
            tmp = p2.enter_context(tc.tile_pool(name="tmp", bufs=2))
            stg = p2.enter_context(tc.tile_pool(name="stg", bufs=2))
            psA = p2.enter_context(tc.tile_pool(name="psA", bufs=3, space="PSUM"))
            psB = p2.enter_context(tc.tile_pool(name="psB", bufs=3, space="PSUM"))
            psT = p2.enter_context(tc.tile_pool(name="psT", bufs=1, space="PSUM"))

            # persistent state (ping-pong on step parity), split per chain
            # so the two chains' dependency cycles stay independent.
            # hTc[c][p]: [128, 16] cols hb*8..+8 = h[c]^T chunk hb
            # c state lives at base partition 32 (rows 32:40) to match the
            # [f|o] gate zone
            hTc = [[st8.tile([128, 2 * R], bf16, name=f"hT{c}{p}",
                             tag=f"hT{c}{p}") for p in range(2)]
                   for c in range(2)]
            cst = [[st8.tile([40, H], f32, name=f"c{c}{p}", tag=f"c{c}{p}")
                    for p in range(2)] for c in range(2)]
            for p in range(2):
                for c in range(2):
                    nc.vector.memset(hTc[c][p], 0.0)
                    nc.vector.memset(cst[c][p], 0.0)

            stt = nc.vector.scalar_tensor_tensor

            def hT_sl(p, c, hb):
                return hTc[c][p][:, hb * R:(hb + 1) * R]

            with tc.For_i(0, T, U, staggered_reset=True) as iv:
                gxch = [gxp.tile([2 * R, U, 512], f32r, name=f"gx{c}",
                                 tag=f"gx{c}") for c in range(2)]
                for c in range(2):
                    nc.sync.dma_start(
                        out=gxch[c],
                        in_=gxst[c][ds(iv, U), :, :].rearrange(
                            "u p g -> p u g"))
                st = [stg.tile([R, U, H], bf16, name=f"st{c}", tag=f"st{c}")
                      for c in range(2)]

                def scatter(c, kk):
                    pool = psA if c == 0 else psB
                    g_ = pool.tile([40, 512], f32, name=f"G{c}", tag=f"G{c}")
                    nc.tensor.matmul(g_, scat_sb, gxch[c][:, kk, :],
                                     start=True, stop=True)
                    return g_

                Gcur = None
                for k in range(U):
                    pp = k % 2
                    if k == 0:
                        Gcur = [scatter(c, 0) for c in range(2)]
                    # prefetch next step's gx into fresh psum banks FIRST:
                    # the PE can run these while the gate matmuls below
                    # still wait on the recurrent state
                    Gnext = ([scatter(c, k + 1) for c in range(2)]
                             if k + 1 < U else None)
                    # gate matmuls accumulate onto gx (h-chunk outer so
                    # the first two only wait on hT chunk 0)
                    for c in range(2):
                        for kc in range(2):
                            for s in range(2):
                                nc.tensor.matmul(
                                    Gcur[c][s * 32:s * 32 + R, :],
                                    hT_sl(pp, c, kc),
                                    whh_sb[c][kc][:, s * 512:(s + 1) * 512],
                                    start=False, stop=True,
                                    skip_group_check=True)
                    # sigmoids (gate cols permuted on host to [i g | f o]:
                    # zone0 rows 0:8 = i|g, zone32 rows 32:40 = f|o)
                    S, slc = [], {}
                    for c in range(2):
                        S_ = spl.tile([40, 512], bf16, name=f"S{c}",
                                      tag=f"S{c}")
                        nc.scalar.activation(S_, Gcur[c], SIG)
                        S.append(S_)
                        slc[c] = (S_[0:R, 0:256], S_[32:32 + R, 0:256],
                                  S_[0:R, 256:512], S_[32:32 + R, 256:512])
                    # c update: GPSIMD takes v = Sf*c (off the critical
                    # cycle); DVE runs each chain's dependent tail
                    # [p, u, cn, h] consecutively so chain A's tail is not
                    # queued behind chain B's ops
                    for c in range(2):
                        Sf = slc[c][1]
                        v_ = tmp.tile([R, H], f32, name=f"v{c}", tag=f"v{c}")
                        nc.gpsimd.tensor_mul(v_, Sf, cst[c][pp][32:40, :])
                        slc[c] = slc[c] + (v_,)
                    for c in range(2):
                        Si, Sf, Sg, So, v_ = slc[c]
                        p_ = tmp.tile([R, H], bf16, name=f"p{c}", tag=f"p{c}")
                        nc.vector.tensor_mul(p_, Si, Sg)
                        u_ = tmp.tile([R, H], f32, name=f"u{c}", tag=f"u{c}")
                        stt(out=u_, in0=p_, scalar=2.0, in1=Si,
                            op0=MULT, op1=SUB)
                        cn = cst[c][1 - pp][32:40, :]
                        nc.vector.tensor_add(cn, u_, v_)
                        tc_ = tmp.tile([40, H], bf16, name=f"tc{c}",
                                       tag=f"tc{c}")
                        nc.scalar.activation(tc_[32:40, :], cn, TANH)
                        # h = sig(o) * tanh(c) -> store buffer (bf16)
                        nc.vector.tensor_mul(st[c][:, k, :], So,
                                             tc_[32:40, :])
                        ptt = psT.tile([128, 2 * R], bf16, name=f"ptt{c}",
                                       tag=f"ptt{c}")
                        # per-chunk transpose+copy: the next step's hb=0
                        # matmuls only wait for chunk 0
                        nc.tensor.transpose(ptt[:, 0:R],
                                            st[c][:, k, 0:128], id8b)
                        nc.scalar.copy(hTc[c][1 - pp][:, 0:R], ptt[:, 0:R])
                        nc.tensor.transpose(ptt[:, R:2 * R],
                                            st[c][:, k, 128:256], id8b)
                        nc.vector.tensor_copy(hTc[c][1 - pp][:, R:2 * R],
                                              ptt[:, R:2 * R])
                    Gcur = Gnext
                for c in range(2):
                    nc.gpsimd.dma_start(
                        out=hst[c][ds(iv, U), :, :].rearrange("u p h -> p u h"),
                        in_=st[c])

        # ------- phase 3: combine, transpose to (b, h, t), interleave -----
        with ExitStack() as p3:
          if not skip3:
            p3c = p3.enter_context(tc.tile_pool(name="p3c", bufs=1))
            lp = p3.enter_context(tc.tile_pool(name="lp", bufs=4))
            cmb = p3.enter_context(tc.tile_pool(name="cmb", bufs=4))
            tp = p3.enter_context(tc.tile_pool(name="tp", bufs=4, space="PSUM"))
            op = p3.enter_context(tc.tile_pool(name="op", bufs=4))

            id128 = p3c.tile([128, 128], f32, tag="id128")
            make_identity(nc, id128)

            for b in range(BL):
                for t in range(TC):
                    tsl = ds(t * 128, 128)
                    a0 = lp.tile([128, H], bf16, tag="a0")
                    b0 = lp.tile([128, H], bf16, tag="b0")
                    a1 = lp.tile([128, H], bf16, tag="a1")
                    b1 = lp.tile([128, H], bf16, tag="b1")
                    nc.sync.dma_start(out=a0, in_=hst[0][tsl, b, :])
                    nc.sync.dma_start(out=b0, in_=hst[1][tsl, b, :])
                    nc.sync.dma_start(out=a1, in_=hst[0][tsl, BL + b, :])
                    nc.sync.dma_start(out=b1, in_=hst[1][tsl, BL + b, :])
                    lr = cmb.tile([128, H], f32, tag="lr")
                    nc.vector.tensor_sub(lr, a0, b0)
                    li = cmb.tile([128, H], f32, tag="li")
                    nc.gpsimd.tensor_add(li, a1, b1)
                    for hb in range(2):
                        hsl = ds(hb * 128, 128)
                        ptr = tp.tile([128, 128], f32, tag="ptr")
                        nc.tensor.transpose(ptr, lr[:, hsl], id128)
                        pti = tp.tile([128, 128], f32, tag="pti")
                        nc.tensor.transpose(pti, li[:, hsl], id128)
                        ot = op.tile([128, 256], f32, tag="ot")
                        otv = ot.rearrange("p (t two) -> p t two", two=2)
                        nc.vector.tensor_copy(otv[:, :, 0], ptr)
                        nc.vector.tensor_copy(otv[:, :, 1], pti)
                        nc.sync.dma_start(
                            out=out[b, hsl, ds(2 * t * 128, 256)], in_=ot)

    nc.compile()
    return nc


_CACHE = {}
LAST_RES = None


def get_program(T):
    if T not in _CACHE:
        _CACHE[T] = build_program(T)
    return _CACHE[T]


def _pack_weights(Wih, Whh, bih, bhh):
    Wih = np.array(Wih, dtype=np.float32, copy=True)
    Whh = np.array(Whh, dtype=np.float32, copy=True)
    b = (np.asarray(bih) + np.asarray(bhh)).astype(np.float32)
    # pre-scale g gate (rows 2H:3H) by 2 so sigmoid(2g) gives tanh via 2s-1
    Wih[2 * H:3 * H] *= 2.0
    Whh[2 * H:3 * H] *= 2.0
    b[2 * H:3 * H] *= 2.0
    # permute gate blocks (i, f, g, o) -> (i, g, f, o) so the kernel's
    # zone0 = [i|g], zone32 = [f|o]
    perm = np.r_[0:H, 2 * H:3 * H, H:2 * H, 3 * H:4 * H]
    Wih = Wih[perm]
    Whh = Whh[perm]
    b = b[perm]
    return np.ascontiguousarray(Wih.T), np.ascontiguousarray(Whh.T), b


def kernel(x_real, x_imag, Wih_r, Whh_r, bih_r, bhh_r,
           Wih_i, Whh_i, bih_i, bhh_i):
    x_real = np.asarray(x_real, dtype=np.float32)
    x_imag = np.asarray(x_imag, dtype=np.float32)
    T = x_real.shape[2]
    nc = get_program(T)

    wihT_r, whhT_r, b_r = _pack_weights(Wih_r, Whh_r, bih_r, bhh_r)
    wihT_i, whhT_i, b_i = _pack_weights(Wih_i, Whh_i, bih_i, bhh_i)
    wih_p = np.ascontiguousarray(np.stack([wihT_r, wihT_i]))
    import ml_dtypes
    whh_p = np.ascontiguousarray(
        np.stack([whhT_r, whhT_i]).astype(ml_dtypes.bfloat16))
    bia_p = np.ascontiguousarray(np.stack([b_r, b_i]))
    scat_p = np.zeros((2 * R, 40), dtype=np.float32)
    for j in range(R):
        scat_p[j, j] = 1.0
        scat_p[R + j, 32 + j] = 1.0

    in_maps = []
    for c in range(NCORES):
        sl = slice(c * BL, (c + 1) * BL)
        in_maps.append({
            "xr": np.ascontiguousarray(x_real[sl]),
            "xi": np.ascontiguousarray(x_imag[sl]),
            "wih": wih_p, "whh": whh_p, "bias": bia_p,
            "scat": scat_p,
        })
    import os
    trace = os.environ.get("K_TRACE") == "1"
    res = run_bass_kernel_spmd(nc, in_maps, list(range(NCORES)), trace=trace)
    global LAST_RES
    LAST_RES = res
    parts = []
    for c in range(NCORES):
        o = np.ascontiguousarray(res.results[c]["out"])  # [BL, H, 2T] f32
        parts.append(o.view(np.complex64))               # [BL, H, T]
    return np.concatenate(parts, axis=0)
